# revision 16
# baseline (speedup 1.0000x reference)
"""DSAttention (de-stationary attention) TRN2 Bass kernel — row-tiled PE.

Computes, per (b, h):
    scores = (q @ k^T) * tau_b + delta_b          [L, S]
    scores = where(causal_mask, -1e9, scores)
    A = softmax(scale * scores)                    (no max-subtraction: logits O(10))
    out = A @ v                                    [L, D]

Strategy: batch*head parallel over 8 cores (4 (b,h) pairs per core).
Per (b,h), transposed-score space, strip-by-strip with the PE array split
into TWO independent 64x128 row tiles (tile_position=(0,0)/(64,0)):
  - QK: contraction is exactly E=64 (the delta bias is applied per-partition
    at exp time, NOT via an aug row), so strip j's XT = KT_j^T @ QT runs on
    one 64-row tile while strip j+1 runs concurrently on the other.  The
    qt/kt data is duplicated onto SBUF partitions 64-127 by an on-device
    SBUF->SBUF DMA so each tile reads its own partition range.
  - exp: p = exp(XT + scale*delta_s) on three engines (ACT true exp LUT,
    DVE + GPSIMD int16 Schraudolph bit-trick, ~±3% sawtooth that mostly
    cancels in the softmax ratio).  Diagonal 128x128 blocks masked with
    affine_select (fill 0).
  - AV: contraction (s within strip, 128) split in half: lower 64 s-rows on
    tile (0,0) accumulate into ot_a, upper 64 on (64,0) into ot_b (separate
    PSUM banks - row tiles may not share a bank), both streaming
    concurrently.  V' has a ones column -> row 64 = softmax denominator.
    Evac: osb = copy(ot_a) (ACT/GPS) + add ot_b (DVE), DMA out.
Host divides numerator rows by the denominator row and un-transposes.

Everything stays in 64x128 PE mode (mode switches drain the array).  PE
floor: (17408 QK + 17408 AV) cols / 2 tiles = 17408 cycles/bh at 2.4 GHz.
The last quarter of the last bh is evacuated in 128-column staircase pieces
(its strips complete column blocks in order) to shorten the kernel tail.
"""

import math
from collections import deque

import numpy as np

import bass_rust
import concourse.bass as bass
import concourse.mybir as mybir
import concourse.tile as tile
from concourse.bass_utils import run_bass_kernel_spmd

B, L, S, H, E, D = 2, 2048, 2048, 16, 64, 64
NCORES = 8
BH = B * H                      # 32 (b,h) pairs
BH_PER_CORE = BH // NCORES      # 4
SCALE = 1.0 / math.sqrt(E)
L_HALF = 1024

F32 = mybir.dt.float32
F16 = mybir.dt.float16
U16 = mybir.dt.uint16

QK_COLS = 2 * L                 # qt | kt, each L wide, 64 rows (dup to 128)
VP_COLS = (S // 128) * (D + 1)  # 1040

# fp16 Schraudolph exp: e = bitcast_fp16(u16(rne(A16*x + B16 + A16*dsc)))
A16 = float(np.float32(1024.0 / np.log(2)))
B16 = 15291.0
LAG = 6                         # software pipeline depth, in strips


def _width(p, j):
    l0 = L_HALF * p
    return min(l0 + L_HALF, L) - max(l0, 128 * j)


def _strips_for_pass(p):
    return list(range(8 if p == 0 else 16))


def _assign_qk_tiles():
    """Balance strip QK columns between the two row tiles, per pass."""
    out = {}
    for p in (0, 1):
        loads = {0: 0, 64: 0}
        for j in sorted(_strips_for_pass(p), key=lambda j: -_width(p, j)):
            ta = 0 if loads[0] <= loads[64] else 64
            out[(p, j)] = ta
            loads[ta] += _width(p, j)
    return out


def _assign_exp_engines():
    """Greedy static balance of per-strip exp across ACT (true exp LUT),
    DVE and GPSIMD (int16 Schraudolph).  Costs in ns; seeds account for
    evac copies (ACT/GPS), evac adds (DVE), diag masks (DVE/GPS split) and
    the ACT table load."""
    units = []
    for p in (0, 1):
        for j in _strips_for_pass(p):
            units.append((p, j, _width(p, j)))
    loads = {"ACT": 325.0 + 4 * 660.0, "DVE": 4 * 540.0}
    cost = {"ACT": (0.833, 262.0), "DVE": (1.042, 170.0)}
    out = {}
    for p, j, w in sorted(units, key=lambda t: -t[2]):
        best = min(("ACT", "DVE"),
                   key=lambda e: loads[e] + cost[e][0] * w + cost[e][1])
        out[(p, j)] = best
        loads[best] += cost[best][0] * w + cost[best][1]
    return out


_QK_TILE = _assign_qk_tiles()
_EXP_ENGINE = _assign_exp_engines()


class _SplitDrainTileContext(tile.TileContext):
    """This walrus build rejects instructions carrying more than one sem
    wait; the kernel-tail drain aggregates one wait per active processor.
    Split them across a chain of drains on the same engine."""

    def _drain_and_barrier(self, tick_clock, wait_clock):
        nc = self.nc
        drain_inst = nc.sync.drain()
        wait_clock.add_sem_waits(
            drain_inst.ins, bass_rust.ScopedClock({None: tick_clock.global_clock})
        )
        si = drain_inst.ins.sync_info
        waits = list(si.on_wait) if si is not None and si.on_wait else []
        if len(waits) > 1:
            si.on_wait = waits[:1]
            for w in waits[1:]:
                d2 = nc.sync.drain()
                d2.ins.sync_info = bass_rust.SyncInfo(on_wait=[w], on_update=[])
        nc.all_engine_barrier()
        popped = nc._tile_sem_poison_stack.pop()
        assert popped is self._sem_poison
        nc.clear_and_free_semaphores(list(self.sems.allocated().values()))
        nc.all_engine_barrier()


def _legalize_waits(nc, max_waits=1):
    """This walrus build rejects instructions with more than `max_waits`
    sem waits. Spill extras onto same-engine NoOps inserted just before
    the offending instruction (same-engine program order preserves the
    wait semantics)."""
    for f in nc.m.functions:
        for bb in f.blocks:
            insts = bb.instructions
            for idx in range(len(insts) - 1, -1, -1):
                inst = insts[idx]
                si = getattr(inst, "sync_info", None)
                if si is None or not si.on_wait:
                    continue
                ow = list(si.on_wait)
                sem = [w for w in ow if w.sync_type == "semaphore"]
                other = [w for w in ow if w.sync_type != "semaphore"]
                budget = max(0, max_waits - len(other))
                if len(sem) <= budget:
                    continue
                keep, spill = sem[:budget], sem[budget:]
                si.on_wait = other + keep
                for w in reversed(spill):
                    n = mybir.InstNoOp(name=f"W-{nc.next_id()}", ins=[], outs=[])
                    n.engine = inst.engine
                    n.sync_info = bass_rust.SyncInfo(on_wait=[w], on_update=[])
                    nc.register_instruction(n, overwrite=True)
                    insts.insert(idx, n)


def _build_program():
    nc = bass.Bass("TRN2", target_bir_lowering=False, debug=False)
    x_d = nc.declare_dram_parameter("x", [BH_PER_CORE, 64, QK_COLS], F16, isOutput=False)
    v_d = nc.declare_dram_parameter("v", [BH_PER_CORE, 128, VP_COLS], F16, isOutput=False)
    d_d = nc.declare_dram_parameter("d", [BH_PER_CORE, 128, 32], F32, isOutput=False)
    # output stays transposed: [bh, quarter, d, l_rel]; row d == D is the
    # softmax denominator; the host divides + un-transposes.
    o_d = nc.declare_dram_parameter("o", [BH_PER_CORE, 4, D + 1, 512], F32, isOutput=True)

    with _SplitDrainTileContext(nc) as tc:
        with (
            tc.tile_pool(name="xin", bufs=1) as in_pool,
            tc.tile_pool(name="p", bufs=LAG + 6) as p_pool,
            tc.tile_pool(name="osb", bufs=6) as osb_pool,
            tc.tile_pool(name="osc", bufs=6) as osc_pool,
            tc.tile_pool(name="xt_ps", bufs=4, space="PSUM") as xt_pool,
            tc.tile_pool(name="ota", bufs=2, space="PSUM") as ota_pool,
            tc.tile_pool(name="otb", bufs=2, space="PSUM") as otb_pool,
        ):
            xbs, vbs, dbs = [], [], []
            for i in range(BH_PER_CORE):
                xb = in_pool.tile([128, QK_COLS], F16, name=f"xb{i}", tag=f"xb{i}")
                vb = in_pool.tile([128, VP_COLS], F16, name=f"vb{i}", tag=f"vb{i}")
                db = in_pool.tile([128, 32], F32, name=f"db{i}", tag=f"db{i}")
                xbs.append(xb)
                vbs.append(vb)
                dbs.append(db)
            # Junk-warmup tile: memset (no DMA dependency) so the PE clock
            # ramp starts the moment the framework preamble ends.
            wt = in_pool.tile([128, 128], F16, name="wt", tag="wt")
            nc.gpsimd.memset(wt, 0.0)
            # bh0's x rides two hardware DMA queues in priority-ordered
            # column-range pieces (lower 64 partitions only); the vector
            # queue mirrors each piece to partitions 64-127 (SBUF->SBUF)
            # for the second row tile.
            nc.gpsimd.dma_start(out=dbs[0], in_=d_d[0])
            for c0, c1 in [(L, L + 512), (L + 512, L + 1024), (1024, L)]:
                nc.sync.dma_start(out=xbs[0][0:64, c0:c1], in_=x_d[0, :, c0:c1])
            for c0, c1 in [(0, 512), (512, 1024), (L + 1024, 2 * L)]:
                nc.scalar.dma_start(out=xbs[0][0:64, c0:c1], in_=x_d[0, :, c0:c1])
            nc.gpsimd.dma_start(out=vbs[0][:, 0:520], in_=v_d[0, :, 0:520])
            # mirror bh0's x pieces to partitions 64-127 (second row tile),
            # most-urgent first
            for c0, c1 in [(L, L + 512), (0, 512), (L + 512, L + 1024),
                           (512, 1024), (1024, L), (L + 1024, 2 * L)]:
                nc.gpsimd.dma_start(out=xbs[0][64:128, c0:c1],
                                    in_=xbs[0][0:64, c0:c1])
            nc.gpsimd.dma_start(out=vbs[0][:, 520:VP_COLS], in_=v_d[0, :, 520:VP_COLS])
            for i in range(1, BH_PER_CORE):
                nc.gpsimd.dma_start(out=dbs[i], in_=d_d[i])
                nc.sync.dma_start(out=xbs[i][0:64, :], in_=x_d[i])
                nc.sync.dma_start(out=xbs[i][64:128, :], in_=xbs[i][0:64, :])
                nc.sync.dma_start(out=vbs[i], in_=v_d[i])

            # pend: software pipeline of strips awaiting their AV matmuls.
            pend = deque()
            pend_evac = deque()
            ota = {}      # (i, q) -> lower-half psum tile
            otb = {}      # (i, q) -> upper-half psum tile
            nunit = 0

            def evac(i, q, c0, c1, eng):
                osc = osc_pool.tile([D + 1, 512], F32, name=f"osc_{i}_{q}_{c0}",
                                    tag="osc")
                osb = osb_pool.tile([D + 1, 512], F32, name=f"osb_{i}_{q}_{c0}",
                                    tag="osb")
                nc.scalar.activation(osc[:, c0:c1], ota[(i, q)][:, c0:c1],
                                     mybir.ActivationFunctionType.Copy)
                nc.vector.tensor_tensor(out=osb[:, c0:c1], in0=osc[:, c0:c1],
                                        in1=otb[(i, q)][:, c0:c1],
                                        op=mybir.AluOpType.add)
                nc.sync.dma_start(out=o_d[i, q, :, c0:c1], in_=osb[:, c0:c1])

            def flush_evacs(min_age):
                while pend_evac and pend_evac[0][0] + min_age <= nunit:
                    _, args = pend_evac.popleft()
                    evac(*args)

            def av_emit(item):
                i, pss, j, p_t, qskip = item
                l0 = L_HALF * pss
                ls = max(l0, 128 * j)
                vb = vbs[i]
                for q in (2 * pss, 2 * pss + 1):
                    ql0 = 512 * q
                    a = max(ql0, ls)
                    if a >= ql0 + 512 or (j, q) in qskip:
                        continue
                    tail = (i == BH_PER_CORE - 1 and q == 3 and j >= 12)
                    for ta, ot in ((0, ota), (64, otb)):
                        nc.tensor.matmul(
                            ot[(i, q)][:, a - ql0:512],
                            lhsT=vb[ta:ta + 64, 65 * j:65 * j + 65],
                            rhs=p_t[ta:ta + 64, a - ls:ql0 + 512 - ls],
                            start=(j == 0), stop=(j == 4 * q + 3),
                            tile_position=(ta, 0),
                            skip_group_check=tail,
                        )
                    if tail:
                        # staircase: strip j completes columns
                        # [128(j-12), 128(j-11)) of the final quarter
                        c0 = 128 * (j - 12)
                        evac(i, q, c0, c0 + 128, "ACT")
                    elif j == 4 * q + 3:
                        pend_evac.append((nunit, (i, q, 0, 512, "ACT")))

            def exp_emit(i, pss, j, p_t, xt, a, b):
                # p[:, a:b] = exp(xt[:, a:b] + scale*delta[strip j])
                eng = _EXP_ENGINE[(pss, j)]
                db = dbs[i]
                if eng == "ACT":
                    nc.scalar.activation(p_t[:, a:b], xt[:, 0:b - a],
                                         mybir.ActivationFunctionType.Exp,
                                         bias=db[:, 2 * j + 1:2 * j + 2])
                else:
                    nc.vector.tensor_scalar(
                        out=p_t[:, a:b].bitcast(U16), in0=xt[:, 0:b - a],
                        scalar1=A16, scalar2=db[:, 2 * j:2 * j + 1],
                        op0=mybir.AluOpType.mult, op1=mybir.AluOpType.add)

            def mask_emit(pss, j, p_t):
                nc.gpsimd.affine_select(
                    out=p_t[:, 0:128], in_=p_t[:, 0:128],
                    compare_op=mybir.AluOpType.is_ge, fill=0.0,
                    base=0, channel_multiplier=-1, pattern=[[1, 128]],
                )

            def qk_emit(i, pss, j, xt, a, b):
                # chunk [a, b) of strip j into its own 1-bank xt tile
                ta = _QK_TILE[(pss, j)]
                xb = xbs[i]
                ls = max(L_HALF * pss, 128 * j)
                nc.tensor.matmul(
                    xt[:, 0:b - a],
                    lhsT=xb[ta:ta + 64, L + 128 * j: L + 128 * j + 128],
                    rhs=xb[ta:ta + 64, ls + a: ls + b],
                    start=True, stop=True, tile_position=(ta, 0),
                )

            def strip_emit(i, pss, j, split):
                # QK -> exp -> mask for one strip, chunked at `split` (<=512
                # per chunk) so each chunk owns a single-bank xt tile and the
                # QK/exp pipeline runs 4 chunks deep.
                nonlocal nunit
                w = _width(pss, j)
                bounds = [0] + ([min(split, w), w] if w > split else [w])
                bounds = sorted(set(bounds))
                p_t = p_pool.tile([128, 1024], F16, name=f"p{nunit}_{i}",
                                  tag="p")
                nunit += 1
                for a, b in zip(bounds[:-1], bounds[1:]):
                    xt = xt_pool.tile([128, 512], F32,
                                      name=f"xt{nunit}_{a}", tag="xt")
                    qk_emit(i, pss, j, xt, a, b)
                    exp_emit(i, pss, j, p_t, xt, a, b)
                    if a == 0 and 128 * j >= L_HALF * pss:
                        mask_emit(pss, j, p_t)
                return p_t

            for i in range(BH_PER_CORE):
                for pss in (0, 1):
                    l0 = L_HALF * pss
                    for q in (2 * pss, 2 * pss + 1):
                        ota[(i, q)] = ota_pool.tile([D + 1, 512], F32,
                                                    name=f"ota_{i}_{q}", tag="ota")
                        otb[(i, q)] = otb_pool.tile([D + 1, 512], F32,
                                                    name=f"otb_{i}_{q}", tag="otb")
                    strips = _strips_for_pass(pss)
                    if i == 0 and pss == 0:
                        # Clock-ramp warm-up: junk matmuls on the memset tile
                        # (no DMA dependency) keep the PE busy from the end of
                        # the framework preamble until real data lands,
                        # rotating over both row tiles and 4 column ranges of
                        # the junk psum banks.
                        for w in range(20):
                            ta = 64 * (w % 2)
                            ota_or_b = ota if w % 2 == 0 else otb
                            c = 128 * ((w // 2) % 4)
                            nc.tensor.matmul(
                                ota_or_b[(0, 1)][:, c:c + 128],
                                lhsT=wt[ta:ta + 64, 0:65],
                                rhs=wt[ta:ta + 64, 0:128],
                                start=True, stop=True, tile_position=(ta, 0),
                                skip_group_check=True)
                        # Warm start: strips 0 and 1 chunk at the qt[0:512]
                        # DMA-piece boundary so their first chunks run on the
                        # first-landed pieces; later chunks follow when
                        # qt[512:1024] lands.
                        for j in (0, 1):
                            p_t = strip_emit(0, 0, j, 512 - 128 * j)
                            pend.append((0, 0, j, p_t, set()))
                        strips = strips[2:]
                    for j in strips:
                        p_t = strip_emit(i, pss, j, 512)
                        pend.append((i, pss, j, p_t, set()))
                        if len(pend) > LAG:
                            av_emit(pend.popleft())
                        flush_evacs(2)
            while pend:
                av_emit(pend.popleft())
                nunit += 1
                flush_evacs(2)
            flush_evacs(0)
    _legalize_waits(nc)
    return nc


_PROGRAM = None


def _get_program():
    global _PROGRAM
    if _PROGRAM is None:
        _PROGRAM = _build_program()
    return _PROGRAM


def _prepare_inputs(q, k, v, tau, delta):
    """Pack full inputs into the per-core fp16 device layout."""
    qs = (q.astype(np.float64) * (SCALE * tau.astype(np.float64))[:, 0, None, None, None]).astype(np.float16)
    # [B,L,H,E] -> [BH, E, L]
    qt = np.ascontiguousarray(qs.transpose(0, 2, 3, 1).reshape(BH, E, L))
    kt = np.ascontiguousarray(k.astype(np.float16).transpose(0, 2, 3, 1).reshape(BH, E, S))
    # V' = [v, 1]: [BH, S, D+1] -> [BH, 128, 16*(D+1)]
    vt = v.astype(np.float16).transpose(0, 2, 1, 3).reshape(BH, S, D)
    vp = np.concatenate([vt, np.ones((BH, S, 1), np.float16)], axis=2)
    vp = np.ascontiguousarray(
        vp.reshape(BH, S // 128, 128, D + 1).transpose(0, 2, 1, 3).reshape(BH, 128, VP_COLS)
    )
    x = np.empty((BH, E, QK_COLS), np.float16)
    x[:, :, 0:L] = qt
    x[:, :, L:2 * L] = kt
    # per-strip exp-bias table: col 2j = B16 + A16*scale*delta (DVE/GPS
    # Schraudolph), col 2j+1 = scale*delta (ACT exp bias)
    dsc = (SCALE * delta.astype(np.float64)).astype(np.float32)  # [B, S]
    dt = np.empty((BH, 128, 32), np.float32)
    for j in range(16):
        blk = dsc[:, 128 * j:128 * j + 128]                      # [B, 128]
        dt[:, :, 2 * j] = np.repeat(B16 + A16 * blk, H, axis=0)
        dt[:, :, 2 * j + 1] = np.repeat(blk, H, axis=0)
    return x, vp, dt


def _numpy_fallback(q, k, v, att_mask, tau, delta):
    out = np.empty((B, L, H, D), np.float32)
    mask = att_mask[:, 0]  # [B, L, S]
    for b in range(B):
        for h in range(H):
            s = (q[b, :, h, :] @ k[b, :, h, :].T) * tau[b, 0] + delta[b][None, :]
            s = np.where(mask[b], -1e9, s).astype(np.float32)
            s = SCALE * s
            s = s - s.max(axis=-1, keepdims=True)
            e = np.exp(s)
            a = e / e.sum(axis=-1, keepdims=True)
            out[b, :, h, :] = a @ v[b, :, h, :]
    return out


def kernel(q, k, v, att_mask, tau, delta):
    q = np.asarray(q, np.float32)
    k = np.asarray(k, np.float32)
    v = np.asarray(v, np.float32)
    tau = np.asarray(tau, np.float32)
    delta = np.asarray(delta, np.float32)
    att_mask = np.asarray(att_mask)

    causal = np.triu(np.ones((L, S), bool), k=1)
    if not all(np.array_equal(att_mask[b, 0], causal) for b in range(B)):
        return _numpy_fallback(q, k, v, att_mask, tau, delta)

    x, vp, dt = _prepare_inputs(q, k, v, tau, delta)
    nc = _get_program()
    in_maps = [
        {
            "x": np.ascontiguousarray(x[c * BH_PER_CORE:(c + 1) * BH_PER_CORE]),
            "v": np.ascontiguousarray(vp[c * BH_PER_CORE:(c + 1) * BH_PER_CORE]),
            "d": np.ascontiguousarray(dt[c * BH_PER_CORE:(c + 1) * BH_PER_CORE]),
        }
        for c in range(NCORES)
    ]
    res = run_bass_kernel_spmd(nc, in_maps, list(range(NCORES))).results

    out = np.empty((B, L, H, D), np.float32)
    for c in range(NCORES):
        o = res[c]["o"]  # [4, 4, D+1, 512]: raw numerators + denominator row
        norm = o[:, :, 0:D, :] / o[:, :, D:D + 1, :]
        for i in range(BH_PER_CORE):
            bh = c * BH_PER_CORE + i
            out[bh // H, :, bh % H, :] = norm[i].transpose(0, 2, 1).reshape(L, D)
    return out


# revision 18
# speedup vs baseline: 1.2113x; 1.2113x over previous
"""DSAttention (de-stationary attention) TRN2 Bass kernel.

Computes, per (b, h):
    scores = (q @ k^T) * tau_b + delta_b          [L, S]
    scores = where(causal_mask, -1e9, scores)
    A = softmax(scale * scores)                    (no max-subtraction: logits O(10))
    out = A @ v                                    [L, D]

Strategy: batch*head parallel over 8 cores (4 (b,h) pairs per core).
Per (b,h), transposed-score space, j-outer over s-strips in two l-passes
(l in [0,1024) then [1024,2048)) so each kt_j / vp_j stationary is loaded
once per pass and causality is exact at 128-col granularity:
    XT_j[s, l] = sum_e KT[e, s] * QT[e, l] (+ aug row: 1.0 * scale*delta[s])
    p_j = exp(XT_j)      split across TWO engines:
          ACT: true exp LUT;  DVE: Schraudolph int16 bit-trick
          (bitcast fp16 approx of 2^(A*x+B), ~+/-3% sawtooth -- softmax
          scale-invariance cancels the mean, only the ripple remains)
    diag 128x128 blocks masked on gpsimd (affine_select, fill 0)
    OT[q][d, l] += V'_j[s, d] * p_j[s, l]  accumulated over j per quarter;
                   V' has a ones column -> row 64 = softmax denominator
Host divides numerator rows by the denominator row and un-transposes.

All matmul inputs are fp16 (measured: K=65 fp16 runs at the full
1 col/cycle PE rate, so the aug-row contraction is free and no pad rows /
memsets are needed). LDWEIGHTS fully overlaps back-to-back matmuls.
PE floor: 2*17408 cols/bh = 34816 cycles/bh at 2.4 GHz.
"""

import math
from collections import deque

import numpy as np

import bass_rust
import concourse.bass as bass
import concourse.mybir as mybir
import concourse.tile as tile
from concourse.bass_utils import run_bass_kernel_spmd

B, L, S, H, E, D = 2, 2048, 2048, 16, 64, 64
NCORES = 8
BH = B * H                      # 32 (b,h) pairs
BH_PER_CORE = BH // NCORES      # 4
SCALE = 1.0 / math.sqrt(E)
L_HALF = 1024

F32 = mybir.dt.float32
F16 = mybir.dt.float16
I16 = mybir.dt.int16

QK_COLS = 2 * L                 # qt | kt, each L wide, 65 rows (64 + aug)
VP_COLS = (S // 128) * (D + 1)  # 1040

# fp16 Schraudolph exp: e = bitcast_fp16(int16(rne(A16*x + B16)))
A16 = float(np.float32(1024.0 / np.log(2)))
B16 = 15291.0
LAG = 3                         # software pipeline depth, in exp-units


def _units_for_pass(p):
    """Strip groups sharing one exp instruction (packed into one PSUM tile)."""
    if p == 0:
        return [[0], [1], [2], [3], [4, 5], [6, 7]]
    return [[j] for j in range(12)] + [[12, 13], [14, 15]]


def _width(p, j):
    l0 = L_HALF * p
    return min(l0 + L_HALF, L) - max(l0, 128 * j)


def _assign_exp_engines():
    """Greedy static balance of exp units between ACT (true exp) and DVE
    (Schraudolph). Costs in ns per unit; loads seeded with the evac copies."""
    units = []
    for p in (0, 1):
        for u in _units_for_pass(p):
            units.append((p, tuple(u), sum(_width(p, j) for j in u)))
    loads = {"ACT": 1200.0, "DVE": 1408.0}
    out = {}
    for p, u, w in sorted(units, key=lambda t: -t[2]):
        ca, cd = 0.833 * w + 262, 1.042 * w + 170
        if loads["ACT"] + ca <= loads["DVE"] + cd:
            out[(p, u)] = "ACT"
            loads["ACT"] += ca
        else:
            out[(p, u)] = "DVE"
            loads["DVE"] += cd
    return out


_EXP_ENGINE = _assign_exp_engines()


class _SplitDrainTileContext(tile.TileContext):
    """This walrus build rejects instructions carrying more than one sem
    wait; the kernel-tail drain aggregates one wait per active processor.
    Split them across a chain of drains on the same engine."""

    def _drain_and_barrier(self, tick_clock, wait_clock):
        nc = self.nc
        drain_inst = nc.sync.drain()
        wait_clock.add_sem_waits(
            drain_inst.ins, bass_rust.ScopedClock({None: tick_clock.global_clock})
        )
        si = drain_inst.ins.sync_info
        waits = list(si.on_wait) if si is not None and si.on_wait else []
        if len(waits) > 1:
            si.on_wait = waits[:1]
            for w in waits[1:]:
                d2 = nc.sync.drain()
                d2.ins.sync_info = bass_rust.SyncInfo(on_wait=[w], on_update=[])
        nc.all_engine_barrier()
        popped = nc._tile_sem_poison_stack.pop()
        assert popped is self._sem_poison
        nc.clear_and_free_semaphores(list(self.sems.allocated().values()))
        nc.all_engine_barrier()


def _legalize_waits(nc, max_waits=1):
    """This walrus build rejects instructions with more than `max_waits`
    sem waits. Spill extras onto same-engine NoOps inserted just before
    the offending instruction (same-engine program order preserves the
    wait semantics)."""
    for f in nc.m.functions:
        for bb in f.blocks:
            insts = bb.instructions
            for idx in range(len(insts) - 1, -1, -1):
                inst = insts[idx]
                si = getattr(inst, "sync_info", None)
                if si is None or not si.on_wait:
                    continue
                ow = list(si.on_wait)
                sem = [w for w in ow if w.sync_type == "semaphore"]
                other = [w for w in ow if w.sync_type != "semaphore"]
                budget = max(0, max_waits - len(other))
                if len(sem) <= budget:
                    continue
                keep, spill = sem[:budget], sem[budget:]
                si.on_wait = other + keep
                for w in reversed(spill):
                    n = mybir.InstNoOp(name=f"W-{nc.next_id()}", ins=[], outs=[])
                    n.engine = inst.engine
                    n.sync_info = bass_rust.SyncInfo(on_wait=[w], on_update=[])
                    nc.register_instruction(n, overwrite=True)
                    insts.insert(idx, n)


def _build_program():
    nc = bass.Bass("TRN2", target_bir_lowering=False, debug=False)
    x_d = nc.declare_dram_parameter("x", [BH_PER_CORE, 65, QK_COLS], F16, isOutput=False)
    v_d = nc.declare_dram_parameter("v", [BH_PER_CORE, 128, VP_COLS], F16, isOutput=False)
    # output stays transposed: [bh, quarter, d, l_rel]; row d == D is the
    # softmax denominator; the host divides + un-transposes.
    o_d = nc.declare_dram_parameter("o", [BH_PER_CORE, 4, D + 1, 512], F32, isOutput=True)

    with _SplitDrainTileContext(nc) as tc:
        with (
            tc.tile_pool(name="xin", bufs=1) as in_pool,
            tc.tile_pool(name="p", bufs=LAG + 3) as p_pool,
            tc.tile_pool(name="osb", bufs=5) as osb_pool,
            tc.tile_pool(name="xt_ps", bufs=3, space="PSUM") as xt_pool,
            tc.tile_pool(name="ot_ps", bufs=2, space="PSUM") as ot_pool,
        ):
            xbs, vbs = [], []
            for i in range(BH_PER_CORE):
                xb = in_pool.tile([65, QK_COLS], F16, name=f"xb{i}", tag=f"xb{i}")
                vb = in_pool.tile([128, VP_COLS], F16, name=f"vb{i}", tag=f"vb{i}")
                xbs.append(xb)
                vbs.append(vb)
            # Tiny warm-up tile: lands ~2us before the first real piece so
            # the PE can ramp its clock on throwaway matmuls instead of
            # idling (full 2.4GHz needs ~3us of continuous execution).
            wt = in_pool.tile([65, 128], F16, name="wt", tag="wt")
            nc.gpsimd.memset(wt, 0.0)
            # bh0's x rides two hardware DMA queues in priority-ordered
            # column-range pieces so the first exp-units' operands (first kt
            # strips + first qt half) land first; aggregate HBM bandwidth is
            # shared by all 8 cores and each queue sustains only ~40 GB/s,
            # so small high-priority pieces — not queue count — shorten the
            # preamble.
            for c0, c1 in [(L, L + 512), (L + 512, L + 1024), (1024, L)]:
                nc.sync.dma_start(out=xbs[0][:, c0:c1], in_=x_d[0, :, c0:c1])
            for c0, c1 in [(0, 512), (512, 1024), (L + 1024, 2 * L)]:
                nc.scalar.dma_start(out=xbs[0][:, c0:c1], in_=x_d[0, :, c0:c1])
            nc.gpsimd.dma_start(out=vbs[0][:, 0:65], in_=v_d[0, :, 0:65])
            nc.gpsimd.dma_start(out=vbs[0][:, 65:VP_COLS], in_=v_d[0, :, 65:VP_COLS])
            for i in range(1, BH_PER_CORE):
                nc.sync.dma_start(out=xbs[i], in_=x_d[i])
                nc.sync.dma_start(out=vbs[i], in_=v_d[i])

            # pend: software pipeline of exp-units awaiting their AV matmuls.
            # Entries carry everything AV needs so the pipeline can run
            # across pass and bh boundaries without draining.
            pend = deque()
            ot = {}       # (i, q) -> psum tile, created lazily per pass
            nunit = 0

            def av_emit(item):
                i, pss, u, p_t, offs, qskip = item
                l0 = L_HALF * pss
                vb = vbs[i]
                for idx, j in enumerate(u):
                    ls = max(l0, 128 * j)
                    off = offs[idx]
                    tail = (i == BH_PER_CORE - 1 and j >= 12)
                    for q in (2 * pss, 2 * pss + 1):
                        ql0 = 512 * q
                        a = max(ql0, ls)
                        if a >= ql0 + 512 or (j, q) in qskip:
                            continue
                        nc.tensor.matmul(
                            ot[(i, q)][:, a - ql0:512],
                            lhsT=vb[:, 65 * j:65 * j + 65],
                            rhs=p_t[:, off + a - ls: off + ql0 + 512 - ls],
                            start=(j == 0), stop=(j == 4 * q + 3),
                            skip_group_check=(tail and q == 3),
                        )
                    if tail:
                        # staircase: strip j completes q3 columns
                        # [128(j-12), 128(j-11)); evacuate immediately so the
                        # kernel tail is one 128-col piece, not a full quarter
                        q = 3
                        c0 = 128 * (j - 12)
                        osb = osb_pool.tile([D + 1, 512], F32,
                                            name=f"osbt_{j}", tag="osb")
                        if j % 2 == 0:
                            nc.vector.tensor_copy(osb[:, c0:c0 + 128],
                                                  ot[(i, q)][:, c0:c0 + 128])
                        else:
                            nc.scalar.activation(
                                osb[:, c0:c0 + 128], ot[(i, q)][:, c0:c0 + 128],
                                mybir.ActivationFunctionType.Copy)
                        nc.sync.dma_start(out=o_d[i, q, :, c0:c0 + 128],
                                          in_=osb[:, c0:c0 + 128])
                    for q in (2 * pss, 2 * pss + 1):
                        if (tail and q == 3) or (j, q) in qskip:
                            continue
                        if j == 4 * q + 3:
                            # quarter complete: evacuate + ship
                            osb = osb_pool.tile([D + 1, 512], F32, name=f"osb_{i}_{q}",
                                                tag="osb")
                            if i == BH_PER_CORE - 1 and q == 3:
                                # terminal quarter: split across both engines
                                # to shorten the kernel tail
                                nc.vector.tensor_copy(osb[:, 0:256],
                                                      ot[(i, q)][:, 0:256])
                                nc.scalar.activation(
                                    osb[:, 256:512], ot[(i, q)][:, 256:512],
                                    mybir.ActivationFunctionType.Copy)
                            elif q % 2 == 0:
                                nc.vector.tensor_copy(osb, ot[(i, q)])
                            else:
                                nc.scalar.activation(
                                    osb, ot[(i, q)],
                                    mybir.ActivationFunctionType.Copy)
                            nc.sync.dma_start(out=o_d[i, q], in_=osb)

            def exp_emit(engine, p_t, xt, a, b):
                if engine == "ACT":
                    nc.scalar.activation(p_t[:, a:b], xt[:, a:b],
                                         mybir.ActivationFunctionType.Exp)
                else:
                    nc.vector.tensor_scalar(
                        out=p_t[:, a:b].bitcast(I16), in0=xt[:, a:b],
                        scalar1=A16, scalar2=B16,
                        op0=mybir.AluOpType.mult, op1=mybir.AluOpType.add)

            def mask_emit(p_t, off):
                nc.gpsimd.affine_select(
                    out=p_t[:, off:off + 128], in_=p_t[:, off:off + 128],
                    compare_op=mybir.AluOpType.is_ge, fill=0.0,
                    base=0, channel_multiplier=-1, pattern=[[1, 128]],
                )

            for i in range(BH_PER_CORE):
                xb = xbs[i]
                for pss in (0, 1):
                    l0 = L_HALF * pss
                    for q in (2 * pss, 2 * pss + 1):
                        ot[(i, q)] = ot_pool.tile([D + 1, 512], F32,
                                                  name=f"ot_{i}_{q}", tag="ot")
                    units = _units_for_pass(pss)
                    if i == 0 and pss == 0:
                        # Warm-start prologue: while qt[512:1024] is still in
                        # flight, run every piece that needs only the first-
                        # landed data (kt strips + qt[0:512] + vp strip 0) —
                        # QK piece-1s of units 0-2 plus unit 0's first-half
                        # exp, mask, and q0 AV — so the PE clock ramps on
                        # continuous work instead of resetting across DMA
                        # stalls.
                        # clock-ramp warm-up: junk matmuls on the early
                        # tile into ot(0,1), rotating over 4 disjoint column
                        # ranges so Tile's subtile WAW deps never chain
                        # consecutive warm-ups; the first real q1 AV
                        # (start=True) zeroes the bank before use
                        for w in range(16):
                            c = 128 * (w % 4)
                            nc.tensor.matmul(
                                ot[(0, 1)][:, c:c + 128], lhsT=wt[:, 0:65],
                                rhs=wt, start=True, stop=True,
                                skip_group_check=True)
                        xts, pts = [], []
                        for k in range(3):
                            xt = xt_pool.tile([128, 1024], F32,
                                              name=f"xtw{k}", tag="xt")
                            p_t = p_pool.tile([128, 1024], F16,
                                              name=f"pw{k}", tag="p")
                            xts.append(xt)
                            pts.append(p_t)
                            nc.tensor.matmul(
                                xt[:, 0:512 - 128 * k],
                                lhsT=xb[:, L + 128 * k: L + 128 * k + 128],
                                rhs=xb[:, 128 * k:512],
                                start=True, stop=True)
                        e0 = _EXP_ENGINE[(0, (0,))]
                        exp_emit(e0, pts[0], xts[0], 0, 512)
                        mask_emit(pts[0], 0)
                        nc.tensor.matmul(
                            ot[(0, 0)][:, 0:512], lhsT=vbs[0][:, 0:65],
                            rhs=pts[0][:, 0:512], start=True, stop=False)
                        # piece-2s (need qt[512:1024]); cut at the tile's
                        # bank boundary where they cross it
                        for k in range(3):
                            w1 = 512 - 128 * k
                            cuts = [w1, 512, 1024 - 128 * k] if k else [512, 1024]
                            for a, b in zip(cuts[:-1], cuts[1:]):
                                nc.tensor.matmul(
                                    xts[k][:, a:b],
                                    lhsT=xb[:, L + 128 * k: L + 128 * k + 128],
                                    rhs=xb[:, 128 * k + a: 128 * k + b],
                                    start=True, stop=True)
                        exp_emit(e0, pts[0], xts[0], 512, 1024)
                        pend.append((0, 0, [0], pts[0], [0], {(0, 0)}))
                        for k in (1, 2):
                            exp_emit(_EXP_ENGINE[(0, (k,))], pts[k], xts[k],
                                     0, 1024 - 128 * k)
                            mask_emit(pts[k], 0)
                            pend.append((0, 0, [k], pts[k], [0], set()))
                        nunit += 3
                        units = units[3:]
                    for u in units:
                        widths = [_width(pss, j) for j in u]
                        offs = [sum(widths[:m]) for m in range(len(u))]
                        wu = sum(widths)
                        xt = xt_pool.tile([128, 1024], F32, name=f"xt{nunit}",
                                          tag="xt")
                        nunit += 1
                        for idx, j in enumerate(u):
                            ls = max(l0, 128 * j)
                            c0, c1 = offs[idx], offs[idx] + widths[idx]
                            # split at the PSUM bank boundary (tile col 512)
                            cuts = [c0, 512, c1] if c0 < 512 < c1 else [c0, c1]
                            for a, b in zip(cuts[:-1], cuts[1:]):
                                nc.tensor.matmul(
                                    xt[:, a:b],
                                    lhsT=xb[:, L + 128 * j: L + 128 * j + 128],
                                    rhs=xb[:, ls + a - c0: ls + b - c0],
                                    start=True, stop=True,
                                )
                        p_t = p_pool.tile([128, 1024], F16, name=f"p{nunit}",
                                          tag="p")
                        if _EXP_ENGINE[(pss, tuple(u))] == "ACT":
                            nc.scalar.activation(
                                p_t[:, 0:wu], xt[:, 0:wu],
                                mybir.ActivationFunctionType.Exp)
                        else:
                            nc.vector.tensor_scalar(
                                out=p_t[:, 0:wu].bitcast(I16), in0=xt[:, 0:wu],
                                scalar1=A16, scalar2=B16,
                                op0=mybir.AluOpType.mult,
                                op1=mybir.AluOpType.add)
                        for idx, j in enumerate(u):
                            if 128 * j >= l0:
                                # diagonal block: zero p where s > l
                                nc.gpsimd.affine_select(
                                    out=p_t[:, offs[idx]:offs[idx] + 128],
                                    in_=p_t[:, offs[idx]:offs[idx] + 128],
                                    compare_op=mybir.AluOpType.is_ge, fill=0.0,
                                    base=0, channel_multiplier=-1,
                                    pattern=[[1, 128]],
                                )
                        pend.append((i, pss, u, p_t, offs, set()))
                        if len(pend) > LAG:
                            av_emit(pend.popleft())
            while pend:
                av_emit(pend.popleft())
    _legalize_waits(nc)
    return nc


_PROGRAM = None


def _get_program():
    global _PROGRAM
    if _PROGRAM is None:
        _PROGRAM = _build_program()
    return _PROGRAM


def _round_f32r(a):
    """Round fp32 to the f32r grid (13 low mantissa bits zeroed, RNE)."""
    b = a.astype(np.float32).view(np.uint32)
    r = (b + np.uint32(0x0FFF) + ((b >> np.uint32(13)) & np.uint32(1))) & ~np.uint32(0x1FFF)
    return r.view(np.float32)


def _prepare_inputs(q, k, v, tau, delta):
    """Pack full inputs into the per-core fp16 device layout."""
    qs = (q.astype(np.float64) * (SCALE * tau.astype(np.float64))[:, 0, None, None, None]).astype(np.float16)
    # [B,L,H,E] -> [BH, E, L]
    qt = np.ascontiguousarray(qs.transpose(0, 2, 3, 1).reshape(BH, E, L))
    kt = np.ascontiguousarray(k.astype(np.float16).transpose(0, 2, 3, 1).reshape(BH, E, S))
    # V' = [v, 1]: [BH, S, D+1] -> [BH, 128, 16*(D+1)]
    vt = v.astype(np.float16).transpose(0, 2, 1, 3).reshape(BH, S, D)
    vp = np.concatenate([vt, np.ones((BH, S, 1), np.float16)], axis=2)
    vp = np.ascontiguousarray(
        vp.reshape(BH, S // 128, 128, D + 1).transpose(0, 2, 1, 3).reshape(BH, 128, VP_COLS)
    )
    dsc = (SCALE * delta).astype(np.float16)  # [B, S]

    x = np.empty((BH, E + 1, QK_COLS), np.float16)
    x[:, 0:E, 0:L] = qt
    x[:, E, 0:L] = 1.0
    x[:, 0:E, L:2 * L] = kt
    x[:, E, L:2 * L] = np.repeat(dsc, H, axis=0)
    return x, vp


def _numpy_fallback(q, k, v, att_mask, tau, delta):
    out = np.empty((B, L, H, D), np.float32)
    mask = att_mask[:, 0]  # [B, L, S]
    for b in range(B):
        for h in range(H):
            s = (q[b, :, h, :] @ k[b, :, h, :].T) * tau[b, 0] + delta[b][None, :]
            s = np.where(mask[b], -1e9, s).astype(np.float32)
            s = SCALE * s
            s = s - s.max(axis=-1, keepdims=True)
            e = np.exp(s)
            a = e / e.sum(axis=-1, keepdims=True)
            out[b, :, h, :] = a @ v[b, :, h, :]
    return out


def kernel(q, k, v, att_mask, tau, delta):
    q = np.asarray(q, np.float32)
    k = np.asarray(k, np.float32)
    v = np.asarray(v, np.float32)
    tau = np.asarray(tau, np.float32)
    delta = np.asarray(delta, np.float32)
    att_mask = np.asarray(att_mask)

    causal = np.triu(np.ones((L, S), bool), k=1)
    if not all(np.array_equal(att_mask[b, 0], causal) for b in range(B)):
        return _numpy_fallback(q, k, v, att_mask, tau, delta)

    x, vp = _prepare_inputs(q, k, v, tau, delta)
    nc = _get_program()
    in_maps = [
        {
            "x": np.ascontiguousarray(x[c * BH_PER_CORE:(c + 1) * BH_PER_CORE]),
            "v": np.ascontiguousarray(vp[c * BH_PER_CORE:(c + 1) * BH_PER_CORE]),
        }
        for c in range(NCORES)
    ]
    res = run_bass_kernel_spmd(nc, in_maps, list(range(NCORES))).results

    out = np.empty((B, L, H, D), np.float32)
    for c in range(NCORES):
        o = res[c]["o"]  # [4, 4, D+1, 512]: raw numerators + denominator row
        norm = o[:, :, 0:D, :] / o[:, :, D:D + 1, :]
        for i in range(BH_PER_CORE):
            bh = c * BH_PER_CORE + i
            out[bh // H, :, bh % H, :] = norm[i].transpose(0, 2, 1).reshape(L, D)
    return out

